# revision 2
# baseline (speedup 1.0000x reference)
"""CooccurrenceEnhancer kernel for Trainium2 (8 NeuronCores, data-parallel).

Computes, for each token row b:
    y[b, :]  = sum_i scores[b, i] * cooc[ids[b, i], :]      (sparse @ dense)
    y[b, ids[b, :]] = -inf                                   (mask existing)
    top-32 (values, indices) of y[b, :]                      (sorted desc)
    output = concat(ids, top_idx), concat(scores, top_vals)

Strategy: batch is sharded across 8 cores (8192 tokens each, 64 tiles of
128).  Per tile: gpsimd.local_scatter builds the sparse score rows in two
fp16 planes (hi/lo split of the fp32 score; cooc is likewise split into
fp16 hi/lo of 256*cooc so all four planes are fp16).  PE transposes the
scatter output and accumulates three fp16 matmuls per K-chunk
(hi*chi + hi*clo + lo*chi) into fp32 PSUM, which reproduces the fp32
matmul to ~1 ulp.  DVE applies the -big candidate mask while copying
PSUM->SBUF, then extracts an exact sorted top-32 with four rounds of
max8 / max_index / match_replace.
"""

import numpy as np
from contextlib import ExitStack

from concourse import bacc, bass, mybir
from concourse import tile
from concourse import library_config
from concourse.bass_utils import run_bass_kernel_spmd

P = 128            # partitions / tokens per tile
E = 512            # number of experts
CAND = 32          # candidates per token
N_CORES = 8
B = 65536          # total tokens
TPC = B // N_CORES  # tokens per core
K_CHUNKS = E // P   # 4
TOPK = 32           # num_to_add = target_size(64) - CAND(32)
ROUNDS = TOPK // 8  # max8 yields 8 per round
MASK_VAL = -60000.0  # fp16-representable, dwarfs |y| <= ~16 after 256x scale
NEG_IMM = -1.0e30    # match_replace fill


def build_nc(ntiles: int = TPC // P):
    """Builds the single-core Bass program (same program runs on all cores)."""
    nc = bacc.Bacc("TRN2", target_bir_lowering=False, debug=False)
    f16 = mybir.dt.float16
    f32 = mybir.dt.float32

    tokens = ntiles * P
    ids_d = nc.dram_tensor("ids16", [tokens, CAND], mybir.dt.int16,
                           kind="ExternalInput").ap()
    shi_d = nc.dram_tensor("shi", [tokens, CAND], f16, kind="ExternalInput").ap()
    slo_d = nc.dram_tensor("slo", [tokens, CAND], f16, kind="ExternalInput").ap()
    chi_d = nc.dram_tensor("chi", [E, E], f16, kind="ExternalInput").ap()
    clo_d = nc.dram_tensor("clo", [E, E], f16, kind="ExternalInput").ap()
    ident_d = nc.dram_tensor("ident", [P, P], f16, kind="ExternalInput").ap()
    vals_d = nc.dram_tensor("out_vals", [tokens, TOPK], f32,
                            kind="ExternalOutput").ap()
    idx_d = nc.dram_tensor("out_idx", [tokens, TOPK], mybir.dt.uint32,
                           kind="ExternalOutput").ap()

    G = 4 if ntiles % 4 == 0 else 1  # tiles per DMA batch group
    ngroups = ntiles // G

    with tile.TileContext(nc) as tc, ExitStack() as ctx:
        const = ctx.enter_context(tc.tile_pool(name="const", bufs=1))
        inp = ctx.enter_context(tc.tile_pool(name="inp", bufs=4))
        scat = ctx.enter_context(tc.tile_pool(name="scat", bufs=9))
        stp = ctx.enter_context(tc.tile_pool(name="stp", bufs=4))
        ysb = ctx.enter_context(tc.tile_pool(name="ysb", bufs=8))
        outp = ctx.enter_context(tc.tile_pool(name="outp", bufs=8))
        psum = ctx.enter_context(tc.tile_pool(name="psum", bufs=4, space="PSUM"))
        pst = ctx.enter_context(tc.tile_pool(name="pst", bufs=3, space="PSUM"))

        nc.gpsimd.load_library(library_config.local_scatter)

        chi_sb = const.tile([P, K_CHUNKS, E], f16)
        clo_sb = const.tile([P, K_CHUNKS, E], f16)
        ident = const.tile([P, P], f16)
        negbig = const.tile([P, CAND], f16)
        for k in range(K_CHUNKS):
            nc.sync.dma_start(out=chi_sb[:, k, :], in_=chi_d[k * P:(k + 1) * P, :])
            nc.sync.dma_start(out=clo_sb[:, k, :], in_=clo_d[k * P:(k + 1) * P, :])
        nc.sync.dma_start(out=ident[:], in_=ident_d[:])
        nc.vector.memset(negbig[:], MASK_VAL)

        for g in range(ngroups):
            grows = slice(g * G * P, (g + 1) * G * P)
            ids_g = inp.tile([P, G, CAND], mybir.dt.int16, tag="ids")
            shi_g = inp.tile([P, G, CAND], f16, tag="shi")
            slo_g = inp.tile([P, G, CAND], f16, tag="slo")
            nc.sync.dma_start(
                out=ids_g[:], in_=ids_d[grows, :].rearrange("(f p) c -> p f c", p=P))
            nc.sync.dma_start(
                out=shi_g[:], in_=shi_d[grows, :].rearrange("(f p) c -> p f c", p=P))
            nc.sync.dma_start(
                out=slo_g[:], in_=slo_d[grows, :].rearrange("(f p) c -> p f c", p=P))

            vals_g = outp.tile([P, G, TOPK], f32, tag="vals")
            idx_g = outp.tile([P, G, TOPK], mybir.dt.uint32, tag="idx")

            for j in range(G):
                ids_t = ids_g[:, j, :]
                s_hi = scat.tile([P, E], f16, tag="s_hi")
                s_lo = scat.tile([P, E], f16, tag="s_lo")
                mask = scat.tile([P, E], f16, tag="mask")
                nc.gpsimd.local_scatter(s_hi[:], shi_g[:, j, :], ids_t,
                                        channels=P, num_elems=E, num_idxs=CAND)
                nc.gpsimd.local_scatter(s_lo[:], slo_g[:, j, :], ids_t,
                                        channels=P, num_elems=E, num_idxs=CAND)
                nc.gpsimd.local_scatter(mask[:], negbig[:], ids_t,
                                        channels=P, num_elems=E, num_idxs=CAND)

                # Transpose the two scatter planes chunk-by-chunk (PE).
                # All 8 transposes pack into one PSUM bank; one wide ACT
                # copy drains them to SBUF (hi chunks even, lo chunks odd).
                st = stp.tile([P, 2 * K_CHUNKS, P], f16, tag="st")
                pt = pst.tile([P, 2 * K_CHUNKS, P], f16, tag="pt")
                for k in range(K_CHUNKS):
                    nc.tensor.transpose(pt[:, 2 * k, :],
                                        s_hi[:, k * P:(k + 1) * P], ident[:])
                    nc.tensor.transpose(pt[:, 2 * k + 1, :],
                                        s_lo[:, k * P:(k + 1) * P], ident[:])
                nc.scalar.copy(st[:], pt[:])

                # y = S_hi @ chi + S_hi @ clo + S_lo @ chi  (fp32 PSUM accum)
                y_ps = psum.tile([P, E], f32, tag="y")
                n_mm = 3 * K_CHUNKS
                mm = 0
                for k in range(K_CHUNKS):
                    for lhsT, rhs in ((st[:, 2 * k, :], chi_sb),
                                      (st[:, 2 * k, :], clo_sb),
                                      (st[:, 2 * k + 1, :], chi_sb)):
                        nc.tensor.matmul(y_ps[:], lhsT, rhs[:, k, :],
                                         start=(mm == 0), stop=(mm == n_mm - 1))
                        mm += 1

                # mask + copy PSUM->SBUF in one DVE pass; topk scans run
                # from SBUF (2x DVE modes)
                y0 = ysb.tile([P, E], f32, tag="y0")
                y1 = ysb.tile([P, E], f32, tag="y1")
                nc.vector.tensor_tensor(out=y0[:], in0=y_ps[:], in1=mask[:],
                                        op=mybir.AluOpType.add)

                v8 = outp.tile([P, TOPK], f32, tag="v8")
                bufs = [y0, y1, y0, y1]
                for r in range(ROUNDS):
                    cur = bufs[r]
                    v_sl = v8[:, r * 8:(r + 1) * 8]
                    nc.vector.max(v_sl, cur[:])
                    nc.vector.max_index(idx_g[:, j, r * 8:(r + 1) * 8], v_sl,
                                        cur[:])
                    if r < ROUNDS - 1:
                        nc.vector.match_replace(bufs[r + 1][:], v_sl, cur[:],
                                                NEG_IMM)

                nc.vector.tensor_scalar_mul(vals_g[:, j, :], v8[:], 1.0 / 256.0)

            nc.scalar.dma_start(
                out=vals_d[grows, :].rearrange("(f p) c -> p f c", p=P),
                in_=vals_g[:])
            nc.scalar.dma_start(
                out=idx_d[grows, :].rearrange("(f p) c -> p f c", p=P),
                in_=idx_g[:])

    nc.compile()
    return nc


def host_prep(candidate_ids, candidate_scores, cooccurrence):
    """Dedup ids per row (summing duplicate scores), fp16-split scores and
    256*cooc.  Returns per-core input maps (plus shared constants)."""
    ids = np.asarray(candidate_ids).astype(np.int32)
    s = np.asarray(candidate_scores).astype(np.float32)
    C = np.asarray(cooccurrence).astype(np.float32)
    nb, cand = ids.shape

    order = np.argsort(ids, axis=1, kind="stable")
    ids_s = np.take_along_axis(ids, order, axis=1)
    s_s = np.take_along_axis(s, order, axis=1)
    first = np.ones_like(ids_s, dtype=bool)
    first[:, 1:] = ids_s[:, 1:] != ids_s[:, :-1]
    grp = np.cumsum(first, axis=1) - 1
    rows = np.repeat(np.arange(nb), cand)
    sums = np.zeros((nb, cand), np.float32)
    np.add.at(sums, (rows, grp.ravel()), s_s.ravel())
    dids = np.full((nb, cand), -1, np.int16)
    rr, cc = np.nonzero(first)
    dids[rr, grp[rr, cc]] = ids_s[rr, cc].astype(np.int16)
    valid = dids >= 0
    sums = np.where(valid, sums, 0).astype(np.float32)

    shi = sums.astype(np.float16)
    slo = (sums - shi.astype(np.float32)).astype(np.float16)
    Cs = (C * np.float32(256.0)).astype(np.float32)
    chi = Cs.astype(np.float16)
    clo = (Cs - chi.astype(np.float32)).astype(np.float16)
    ident = np.eye(P, dtype=np.float16)

    in_maps = []
    for c in range(N_CORES):
        sh = slice(c * TPC, (c + 1) * TPC)
        in_maps.append({
            "ids16": np.ascontiguousarray(dids[sh]),
            "shi": np.ascontiguousarray(shi[sh]),
            "slo": np.ascontiguousarray(slo[sh]),
            "chi": chi,
            "clo": clo,
            "ident": ident,
        })
    return in_maps


_NC_CACHE = {}


def _get_nc(ntiles):
    if ntiles not in _NC_CACHE:
        _NC_CACHE[ntiles] = build_nc(ntiles)
    return _NC_CACHE[ntiles]


def run_device(in_maps, trace=False, ntiles=TPC // P):
    nc = _get_nc(ntiles)
    return run_bass_kernel_spmd(nc, in_maps, list(range(len(in_maps))),
                                trace=trace)


def kernel(candidate_ids, candidate_scores, cooccurrence, target_size):
    ids = np.asarray(candidate_ids)
    s = np.asarray(candidate_scores).astype(np.float32)
    in_maps = host_prep(ids, s, cooccurrence)
    br = run_device(in_maps)
    vals = np.concatenate([br.results[c]["out_vals"] for c in range(N_CORES)], 0)
    idx = np.concatenate([br.results[c]["out_idx"] for c in range(N_CORES)], 0)
    add_ids = idx.view(np.int32).astype(ids.dtype)
    expanded_ids = np.concatenate([ids, add_ids], axis=1)
    expanded_scores = np.concatenate([s, vals], axis=1)
    return expanded_ids, expanded_scores



# revision 3
# speedup vs baseline: 1.0862x; 1.0862x over previous
"""CooccurrenceEnhancer kernel for Trainium2 (8 NeuronCores, data-parallel).

Computes, for each token row b:
    y[b, :]  = sum_i scores[b, i] * cooc[ids[b, i], :]      (sparse @ dense)
    y[b, ids[b, :]] = -big                                   (mask existing)
    top-32 (values, indices) of y[b, :]                      (sorted desc)
    output = concat(ids, top_idx), concat(scores, top_vals)

Strategy: batch sharded across 8 cores (8192 tokens each, 64 tiles of 128).
Per tile: gpsimd.local_scatter builds sparse score rows in two fp16 planes
(hi/lo split of fp32 scores; cooc split into fp16 hi/lo of 256*cooc).  PE
transposes the scatter planes and accumulates three fp16 matmuls per K-chunk
(hi*chi + hi*clo + lo*chi) into fp32 PSUM (~1 ulp of the fp32 matmul), then
one extra identity-matmul streams the scattered -60000 mask plane into the
same PSUM bank (candidate masking with zero DVE cost).  ACT drains PSUM to
SBUF quickly (frees the bank for the next tile's matmuls without waiting on
the DVE top-k chain).  DVE extracts the exact sorted top-32 with four rounds
of max8 / max_index / match_replace from SBUF; ACT applies the final 1/256
de-scale while casting out.
"""

import numpy as np
from contextlib import ExitStack

from concourse import bacc, bass, mybir
from concourse import tile
from concourse import library_config
from concourse.bass_utils import run_bass_kernel_spmd

P = 128            # partitions / tokens per tile
E = 512            # number of experts
CAND = 32          # candidates per token
N_CORES = 8
B = 65536          # total tokens
TPC = B // N_CORES  # tokens per core
K_CHUNKS = E // P   # 4
TOPK = 32           # num_to_add = target_size(64) - CAND(32)
ROUNDS = TOPK // 8  # max8 yields 8 per round
MASK_VAL = -60000.0  # fp16-representable, dwarfs |y| <= ~16 after 256x scale
NEG_IMM = -1.0e30    # match_replace fill


def build_nc(ntiles: int = TPC // P):
    """Builds the single-core Bass program (same program runs on all cores)."""
    nc = bacc.Bacc("TRN2", target_bir_lowering=False, debug=False)
    f16 = mybir.dt.float16
    f32 = mybir.dt.float32

    tokens = ntiles * P
    ids_d = nc.dram_tensor("ids16", [tokens, CAND], mybir.dt.int16,
                           kind="ExternalInput").ap()
    shi_d = nc.dram_tensor("shi", [tokens, CAND], f16, kind="ExternalInput").ap()
    slo_d = nc.dram_tensor("slo", [tokens, CAND], f16, kind="ExternalInput").ap()
    chi_d = nc.dram_tensor("chi", [E, E], f16, kind="ExternalInput").ap()
    clo_d = nc.dram_tensor("clo", [E, E], f16, kind="ExternalInput").ap()
    ident_d = nc.dram_tensor("ident", [P, P], f16, kind="ExternalInput").ap()
    vals_d = nc.dram_tensor("out_vals", [tokens, TOPK], f32,
                            kind="ExternalOutput").ap()
    idx_d = nc.dram_tensor("out_idx", [tokens, TOPK], mybir.dt.uint32,
                           kind="ExternalOutput").ap()

    G = 4 if ntiles % 4 == 0 else 1  # tiles per DMA batch group
    ngroups = ntiles // G

    with tile.TileContext(nc) as tc, ExitStack() as ctx:
        const = ctx.enter_context(tc.tile_pool(name="const", bufs=1))
        inp = ctx.enter_context(tc.tile_pool(name="inp", bufs=3))
        scat = ctx.enter_context(tc.tile_pool(name="scat", bufs=8))
        stp = ctx.enter_context(tc.tile_pool(name="stp", bufs=4))
        ysb = ctx.enter_context(tc.tile_pool(name="ysb", bufs=8))
        outp = ctx.enter_context(tc.tile_pool(name="outp", bufs=6))
        psum = ctx.enter_context(tc.tile_pool(name="psum", bufs=4, space="PSUM"))
        pst = ctx.enter_context(tc.tile_pool(name="pst", bufs=3, space="PSUM"))

        nc.gpsimd.load_library(library_config.local_scatter)

        chi_sb = const.tile([P, K_CHUNKS, E], f16)
        clo_sb = const.tile([P, K_CHUNKS, E], f16)
        ident = const.tile([P, P], f16)
        negbig = const.tile([P, CAND], f16)
        for k in range(K_CHUNKS):
            nc.sync.dma_start(out=chi_sb[:, k, :], in_=chi_d[k * P:(k + 1) * P, :])
            nc.sync.dma_start(out=clo_sb[:, k, :], in_=clo_d[k * P:(k + 1) * P, :])
        nc.sync.dma_start(out=ident[:], in_=ident_d[:])
        nc.vector.memset(negbig[:], MASK_VAL)

        for g in range(ngroups):
            grows = slice(g * G * P, (g + 1) * G * P)
            ids_g = inp.tile([P, G, CAND], mybir.dt.int16, tag="ids")
            shi_g = inp.tile([P, G, CAND], f16, tag="shi")
            slo_g = inp.tile([P, G, CAND], f16, tag="slo")
            nc.sync.dma_start(
                out=ids_g[:], in_=ids_d[grows, :].rearrange("(f p) c -> p f c", p=P))
            nc.sync.dma_start(
                out=shi_g[:], in_=shi_d[grows, :].rearrange("(f p) c -> p f c", p=P))
            nc.sync.dma_start(
                out=slo_g[:], in_=slo_d[grows, :].rearrange("(f p) c -> p f c", p=P))

            vals_g = outp.tile([P, G, TOPK], f32, tag="vals")
            idx_g = outp.tile([P, G, TOPK], mybir.dt.uint32, tag="idx")

            for j in range(G):
                ids_t = ids_g[:, j, :]
                s_hi = scat.tile([P, E], f16, tag="s_hi")
                s_lo = scat.tile([P, E], f16, tag="s_lo")
                mask = scat.tile([P, E], f16, tag="mask")
                nc.gpsimd.local_scatter(s_hi[:], shi_g[:, j, :], ids_t,
                                        channels=P, num_elems=E, num_idxs=CAND)
                nc.gpsimd.local_scatter(s_lo[:], slo_g[:, j, :], ids_t,
                                        channels=P, num_elems=E, num_idxs=CAND)
                nc.gpsimd.local_scatter(mask[:], negbig[:], ids_t,
                                        channels=P, num_elems=E, num_idxs=CAND)

                # Transpose the two scatter planes chunk-by-chunk (PE).
                # All 8 transposes pack into one PSUM bank; one wide ACT
                # copy drains them to SBUF (hi chunks even, lo chunks odd).
                st = stp.tile([P, 2 * K_CHUNKS, P], f16, tag="st")
                pt = pst.tile([P, 2 * K_CHUNKS, P], f16, tag="pt")
                for k in range(K_CHUNKS):
                    nc.tensor.transpose(pt[:, 2 * k, :],
                                        s_hi[:, k * P:(k + 1) * P], ident[:])
                    nc.tensor.transpose(pt[:, 2 * k + 1, :],
                                        s_lo[:, k * P:(k + 1) * P], ident[:])
                nc.scalar.copy(st[:], pt[:])

                # y = S_hi @ chi + S_hi @ clo + S_lo @ chi  (fp32 PSUM accum)
                # + ident.T @ mask  (adds -60000 at candidate columns)
                y_ps = psum.tile([P, E], f32, tag="y")
                mm = 0
                for k in range(K_CHUNKS):
                    for lhsT, rhs in ((st[:, 2 * k, :], chi_sb[:, k, :]),
                                      (st[:, 2 * k, :], clo_sb[:, k, :]),
                                      (st[:, 2 * k + 1, :], chi_sb[:, k, :])):
                        nc.tensor.matmul(y_ps[:], lhsT, rhs,
                                         start=(mm == 0), stop=False)
                        mm += 1
                nc.tensor.matmul(y_ps[:], ident[:], mask[:],
                                 start=False, stop=True)

                # fast PSUM->SBUF drain on ACT (frees the bank; the DVE
                # top-k chain lags behind in SBUF)
                y0 = ysb.tile([P, E], f32, tag="y0")
                y1 = ysb.tile([P, E], f32, tag="y1")
                nc.scalar.copy(y0[:], y_ps[:])

                v8 = outp.tile([P, TOPK], f32, tag="v8")
                bufs = [y0, y1, y0, y1]
                for r in range(ROUNDS):
                    cur = bufs[r]
                    v_sl = v8[:, r * 8:(r + 1) * 8]
                    nc.vector.max(v_sl, cur[:])
                    nc.vector.max_index(idx_g[:, j, r * 8:(r + 1) * 8], v_sl,
                                        cur[:])
                    if r < ROUNDS - 1:
                        nc.vector.match_replace(bufs[r + 1][:], v_sl, cur[:],
                                                NEG_IMM)

                nc.scalar.activation(vals_g[:, j, :], v8[:],
                                     mybir.ActivationFunctionType.Copy,
                                     scale=1.0 / 256.0)

            nc.scalar.dma_start(
                out=vals_d[grows, :].rearrange("(f p) c -> p f c", p=P),
                in_=vals_g[:])
            nc.scalar.dma_start(
                out=idx_d[grows, :].rearrange("(f p) c -> p f c", p=P),
                in_=idx_g[:])

    nc.compile()
    return nc


def host_prep(candidate_ids, candidate_scores, cooccurrence):
    """Dedup ids per row (summing duplicate scores), fp16-split scores and
    256*cooc.  Returns per-core input maps (plus shared constants)."""
    ids = np.asarray(candidate_ids).astype(np.int32)
    s = np.asarray(candidate_scores).astype(np.float32)
    C = np.asarray(cooccurrence).astype(np.float32)
    nb, cand = ids.shape

    order = np.argsort(ids, axis=1, kind="stable")
    ids_s = np.take_along_axis(ids, order, axis=1)
    s_s = np.take_along_axis(s, order, axis=1)
    first = np.ones_like(ids_s, dtype=bool)
    first[:, 1:] = ids_s[:, 1:] != ids_s[:, :-1]
    grp = np.cumsum(first, axis=1) - 1
    rows = np.repeat(np.arange(nb), cand)
    sums = np.zeros((nb, cand), np.float32)
    np.add.at(sums, (rows, grp.ravel()), s_s.ravel())
    dids = np.full((nb, cand), -1, np.int16)
    rr, cc = np.nonzero(first)
    dids[rr, grp[rr, cc]] = ids_s[rr, cc].astype(np.int16)
    valid = dids >= 0
    sums = np.where(valid, sums, 0).astype(np.float32)

    shi = sums.astype(np.float16)
    slo = (sums - shi.astype(np.float32)).astype(np.float16)
    Cs = (C * np.float32(256.0)).astype(np.float32)
    chi = Cs.astype(np.float16)
    clo = (Cs - chi.astype(np.float32)).astype(np.float16)
    ident = np.eye(P, dtype=np.float16)

    in_maps = []
    for c in range(N_CORES):
        sh = slice(c * TPC, (c + 1) * TPC)
        in_maps.append({
            "ids16": np.ascontiguousarray(dids[sh]),
            "shi": np.ascontiguousarray(shi[sh]),
            "slo": np.ascontiguousarray(slo[sh]),
            "chi": chi,
            "clo": clo,
            "ident": ident,
        })
    return in_maps


_NC_CACHE = {}


def _get_nc(ntiles):
    if ntiles not in _NC_CACHE:
        _NC_CACHE[ntiles] = build_nc(ntiles)
    return _NC_CACHE[ntiles]


def run_device(in_maps, trace=False, ntiles=TPC // P):
    nc = _get_nc(ntiles)
    return run_bass_kernel_spmd(nc, in_maps, list(range(len(in_maps))),
                                trace=trace)


def kernel(candidate_ids, candidate_scores, cooccurrence, target_size):
    ids = np.asarray(candidate_ids)
    s = np.asarray(candidate_scores).astype(np.float32)
    in_maps = host_prep(ids, s, cooccurrence)
    br = run_device(in_maps)
    vals = np.concatenate([br.results[c]["out_vals"] for c in range(N_CORES)], 0)
    idx = np.concatenate([br.results[c]["out_idx"] for c in range(N_CORES)], 0)
    add_ids = idx.view(np.int32).astype(ids.dtype)
    expanded_ids = np.concatenate([ids, add_ids], axis=1)
    expanded_scores = np.concatenate([s, vals], axis=1)
    return expanded_ids, expanded_scores
